# revision 5
# baseline (speedup 1.0000x reference)
"""Fused AllReduce + residual-add + RMSNorm for TRN2 (8 NeuronCores).

Problem: input [8, 8192, 4096] f32 (8 simulated TP ranks), residual
[8192, 4096], norm_weight [4096].  reference = sum(input, axis=0) +
residual, then RMSNorm with gamma; returns (out, residual_out).

Sharding choice: instead of giving each core one rank shard and paying a
wire-level collective (~N bytes/rank over NeuronLink), we shard the TOKEN
axis: core j holds rows [j*1024, (j+1)*1024) of ALL 8 rank shards and sums
them locally.  Zero inter-core traffic; each element is read from HBM
exactly once, which makes the kernel purely HBM-bound (~185 MB/core at
~358 GB/s/core).
"""

import numpy as np

import concourse.bass as bass
import concourse.tile as tile
from concourse import bacc, mybir
from concourse.bass_utils import run_bass_kernel_spmd

TP = 8          # simulated tensor-parallel ranks (leading axis of input)
T = 8192        # tokens
H = 4096        # hidden
NCORES = 8
ROWS = T // NCORES     # 1024 token rows per core
RT = 128               # row tile = SBUF partition count
NT = ROWS // RT        # 8 row tiles per core
HALF = H // 2          # 2048: half-hidden column chunk (1 MB tiles)
EPS = 1e-6

_FP32 = mybir.dt.float32


def _build_nc() -> bass.Bass:
    # Bacc (not raw Bass): its compile() pass legalizes multi-semaphore
    # waits via event semaphores — walrus rejects >1 sync wait on most
    # compute ISA structs otherwise.
    nc = bacc.Bacc("TRN2", target_bir_lowering=False, debug=False, num_devices=NCORES)

    x = nc.declare_dram_parameter("x", [TP, ROWS, H], _FP32, isOutput=False)
    res = nc.declare_dram_parameter("res", [ROWS, H], _FP32, isOutput=False)
    w = nc.declare_dram_parameter("w", [RT, H], _FP32, isOutput=False)
    out = nc.declare_dram_parameter("out", [ROWS, H], _FP32, isOutput=True)
    rout = nc.declare_dram_parameter("rout", [ROWS, H], _FP32, isOutput=True)

    with tile.TileContext(nc) as tc:
        with (
            tc.tile_pool(name="wpool", bufs=1) as wpool,
            tc.tile_pool(name="inp", bufs=10) as inp,
            tc.tile_pool(name="accp", bufs=2) as accp,
            tc.tile_pool(name="sqp", bufs=2) as sqp,
            tc.tile_pool(name="outp", bufs=4) as outp,
            tc.tile_pool(name="statp", bufs=4) as statp,
        ):
            wtile = wpool.tile([RT, H], _FP32)
            nc.sync.dma_start(wtile[:], w[:, :])

            for t in range(NT):
                r0 = t * RT
                acc = accp.tile([RT, H], _FP32, tag="acc")

                # binary-tree rank reduction per column half: each
                # instruction depends on at most two producers, keeping the
                # codegen sync-wait count within ISA limits.
                for h in range(2):
                    cs = slice(h * HALF, (h + 1) * HALF)
                    tiles = []
                    for r in range(TP):
                        b = inp.tile([RT, HALF], _FP32, tag="inb")
                        nc.sync.dma_start(b[:], x[r, r0 : r0 + RT, cs])
                        tiles.append(b)
                    rtile = inp.tile([RT, HALF], _FP32, tag="inb")
                    nc.sync.dma_start(rtile[:], res[r0 : r0 + RT, cs])

                    while len(tiles) > 2:
                        nxt = []
                        for k in range(0, len(tiles), 2):
                            nc.vector.tensor_add(
                                tiles[k][:], tiles[k][:], tiles[k + 1][:]
                            )
                            nxt.append(tiles[k])
                        tiles = nxt
                    nc.vector.tensor_add(tiles[0][:], tiles[0][:], tiles[1][:])
                    # final: sum(ranks) + residual -> acc half
                    nc.vector.tensor_add(acc[:, cs], tiles[0][:], rtile[:])

                # acc is now residual_out for these 128 rows
                nc.sync.dma_start(rout[r0 : r0 + RT, :], acc[:, :])

                # RMSNorm: var = mean(acc^2, -1); out = acc * rsqrt(var+eps) * w
                ss = statp.tile([RT, 2], _FP32, tag="ss")
                for h in range(2):
                    cs = slice(h * HALF, (h + 1) * HALF)
                    sq = sqp.tile([RT, HALF], _FP32, tag="sq")
                    nc.scalar.activation(
                        sq[:],
                        acc[:, cs],
                        mybir.ActivationFunctionType.Square,
                        accum_out=ss[:, h : h + 1],
                    )
                s1 = statp.tile([RT, 1], _FP32, tag="s1")
                nc.vector.tensor_reduce(
                    s1[:], ss[:], axis=mybir.AxisListType.X, op=mybir.AluOpType.add
                )
                ve = statp.tile([RT, 1], _FP32, tag="ve")
                nc.vector.tensor_scalar(
                    ve[:],
                    s1[:],
                    1.0 / H,
                    EPS,
                    op0=mybir.AluOpType.mult,
                    op1=mybir.AluOpType.add,
                )
                ri = statp.tile([RT, 1], _FP32, tag="ri")
                nc.vector.reciprocal(ri[:], ve[:])
                rs = statp.tile([RT, 1], _FP32, tag="rs")
                nc.scalar.sqrt(rs[:], ri[:])

                for h in range(2):
                    cs = slice(h * HALF, (h + 1) * HALF)
                    o = outp.tile([RT, HALF], _FP32, tag="ot")
                    nc.vector.scalar_tensor_tensor(
                        o[:],
                        acc[:, cs],
                        rs[:, 0:1],
                        wtile[:, cs],
                        op0=mybir.AluOpType.mult,
                        op1=mybir.AluOpType.mult,
                    )
                    nc.sync.dma_start(out[r0 : r0 + RT, cs], o[:])

    nc.compile()
    return nc


_NC_CACHE: dict[str, bass.Bass] = {}


def _get_nc() -> bass.Bass:
    if "nc" not in _NC_CACHE:
        _NC_CACHE["nc"] = _build_nc()
    return _NC_CACHE["nc"]


def _make_in_maps(input, residual, norm_weight):
    inp = np.asarray(input, dtype=np.float32)
    res = np.asarray(residual, dtype=np.float32)
    w = np.asarray(norm_weight, dtype=np.float32)
    wt = np.ascontiguousarray(np.broadcast_to(w[None, :], (RT, H)))
    in_maps = []
    for j in range(NCORES):
        sl = slice(j * ROWS, (j + 1) * ROWS)
        in_maps.append(
            {
                "x": np.ascontiguousarray(inp[:, sl, :]),
                "res": np.ascontiguousarray(res[sl]),
                "w": wt,
            }
        )
    return in_maps


def run(input, residual, norm_weight, **spmd_kwargs):
    """Build + run; returns (out, residual_out, BassKernelResults)."""
    nc = _get_nc()
    in_maps = _make_in_maps(input, residual, norm_weight)
    r = run_bass_kernel_spmd(nc, in_maps, core_ids=list(range(NCORES)), **spmd_kwargs)
    out = np.concatenate([r.results[j]["out"] for j in range(NCORES)], axis=0)
    rout = np.concatenate([r.results[j]["rout"] for j in range(NCORES)], axis=0)
    return out, rout, r


def kernel(input, residual, norm_weight):
    out, rout, _ = run(input, residual, norm_weight)
    return out, rout
